# revision 8
# baseline (speedup 1.0000x reference)
"""BiLSTM (3-layer, custom 3-gate cell, highway) Trainium2 Bass kernel.

Sharding: data-parallel over batch B=32 -> 4 per core across 8 cores.
Each core runs the full 3-layer bidirectional scan on its batch shard.

Layouts (per core, B4 = 4 batch lanes):
  - Transposed activations: [128 partitions, chunk, T*B4] with the feature
    dim on partitions (chunks of 128) and (t, b) in the free dim, b inner.
  - Recurrent matmuls are weight-stationary: lhsT = Wh chunk [K=128, M=128],
    rhs = h^T [128, B4] -> psum z^T [128 (z-chunk), B4].
  - Gate column chunks permuted host-side to [i0 i1 o0 o1 j0 j1] so sigmoid
    covers one contiguous [128, 4, B4] slab and tanh covers [128, 2, B4].
"""

import numpy as np

T, BFULL, DIN, H, L = 1024, 32, 256, 256, 3
NCORES = 8
B4 = BFULL // NCORES          # 4 batch lanes per core
TB = T * B4                   # 4096 free-dim elements
S = 32                        # timesteps per For_i iteration
NIT = T // S                  # 32 iterations
NBLK = TB // 512              # 8 blocks of 512 for dense matmuls

_CACHE = {}
TRACE = False


def _build():
    import concourse.bass as bass
    import concourse.bacc as bacc
    import concourse.mybir as mybir
    import concourse.tile as tile

    fp32 = mybir.dt.float32
    AF = mybir.ActivationFunctionType

    nc = bacc.Bacc("TRN2", target_bir_lowering=False, debug=False,
                   num_devices=NCORES)

    # ---------------- I/O declarations ----------------
    xt = nc.dram_tensor("xt", [128, 2, TB], fp32, kind="ExternalInput")
    h0t = nc.dram_tensor("h0t", [128, 2, B4], fp32, kind="ExternalInput")
    c0t = nc.dram_tensor("c0t", [128, 2, B4], fp32, kind="ExternalInput")
    wx_in, wh_in, bias_in = {}, {}, {}
    for l in range(L):
        kc = 2 if l == 0 else 4
        for d in range(2):
            wx_in[(l, d)] = nc.dram_tensor(f"wx{l}{d}", [128, kc, 768], fp32, kind="ExternalInput")
            wh_in[(l, d)] = nc.dram_tensor(f"wh{l}{d}", [128, 2, 768], fp32, kind="ExternalInput")
            bias_in[(l, d)] = nc.dram_tensor(f"bias{l}{d}", [128, 6], fp32, kind="ExternalInput")
    whw_in = nc.dram_tensor("whw", [128, 4, 512], fp32, kind="ExternalInput")
    bhw_in = nc.dram_tensor("bhw", [128, 4], fp32, kind="ExternalInput")

    out2 = nc.dram_tensor("out2", [128, 4, TB], fp32, kind="ExternalOutput")
    hn = nc.dram_tensor("hn", [128, 6, 2, B4], fp32, kind="ExternalOutput")
    cn = nc.dram_tensor("cn", [128, 6, 2, B4], fp32, kind="ExternalOutput")

    from contextlib import ExitStack
    ctx = ExitStack()
    with tile.TileContext(nc) as tc, ctx:
        wpool = ctx.enter_context(tc.tile_pool(name="weights", bufs=1))
        state = ctx.enter_context(tc.tile_pool(name="state", bufs=1))
        io = ctx.enter_context(tc.tile_pool(name="io", bufs=2))
        work = ctx.enter_context(tc.tile_pool(name="work", bufs=4))
        psum = ctx.enter_context(tc.tile_pool(name="psum", bufs=2, space="PSUM"))
        dram = ctx.enter_context(tc.tile_pool(name="dram", bufs=1, space="DRAM"))

        # DRAM scratch (distinct tags -> no aliasing; deps tracked via pool)
        zx_t = {(l, d): dram.tile([128, 6, TB], fp32, tag=f"zx{l}{d}", name=f"zx{l}{d}")
                for l in range(L) for d in range(2)}
        yr = [dram.tile([128, 4, TB], fp32, tag=f"yr{l}", name=f"yr{l}") for l in range(L)]
        ocur = dram.tile([128, 4, TB], fp32, tag="ocur")

        # ------------- persistent state tiles -------------
        ystg = [state.tile([128, 2, (S + 1) * B4], fp32, tag=f"ystg{d}", name=f"ystg{d}") for d in range(2)]
        cA = [state.tile([128, 2, B4], fp32, tag=f"cA{d}", name=f"cA{d}") for d in range(2)]
        cB = [state.tile([128, 2, B4], fp32, tag=f"cB{d}", name=f"cB{d}") for d in range(2)]
        bias_sb = {}
        for l in range(L):
            for d in range(2):
                t = wpool.tile([128, 6], fp32, tag=f"bias{l}{d}")
                nc.gpsimd.dma_start(out=t, in_=bias_in[(l, d)][:, :])
                bias_sb[(l, d)] = t
        bhw_sb = wpool.tile([128, 4], fp32, tag="bhw")
        nc.gpsimd.dma_start(out=bhw_sb, in_=bhw_in[:, :])
        whw_sb = wpool.tile([128, 4, 512], fp32, tag="whw")
        nc.gpsimd.dma_start(out=whw_sb, in_=whw_in[:, :, :])

        def precompute(l, src):
            """zx[l][d][:, m, :] = (Wx_d^T @ src)[m-chunk] + bias."""
            kc = 2 if l == 0 else 4
            wx_sb = [wpool.tile([128, 4, 768], fp32, tag=f"wx{d}", name=f"wx{d}") for d in range(2)]
            for d in range(2):
                nc.gpsimd.dma_start(out=wx_sb[d][:, 0:kc, :], in_=wx_in[(l, d)][:, :, :])
            for nb in range(NBLK):
                xblk = io.tile([128, 4, 512], fp32, tag="xblk")
                nc.gpsimd.dma_start(out=xblk[:, 0:kc, :],
                                  in_=src[:, 0:kc, nb * 512:(nb + 1) * 512])
                for d in range(2):
                    for m in range(6):
                        ps = psum.tile([128, 512], fp32, tag="ps_big")
                        for k in range(kc):
                            nc.tensor.matmul(
                                ps,
                                lhsT=wx_sb[d][:, k, m * 128:(m + 1) * 128],
                                rhs=xblk[:, k, :],
                                start=(k == 0),
                                stop=(k == kc - 1),
                            )
                        zs = work.tile([128, 512], fp32, tag="zs_pre")
                        nc.scalar.activation(
                            out=zs, in_=ps, func=AF.Identity,
                            bias=bias_sb[(l, d)][:, m:m + 1],
                        )
                        nc.gpsimd.dma_start(
                            out=zx_t[(l, d)][:, m, nb * 512:(nb + 1) * 512], in_=zs)

        def step(d, s, zx_sb, wh_sb):
            """One timestep of direction d (python-static within For_i body)."""
            stg = ystg[d]
            if d == 0:
                rd_slot, wr_slot, u = s, s + 1, s
            else:
                rd_slot, wr_slot, u = S - s, S - 1 - s, S - 1 - s
            h_rhs = stg[:, :, rd_slot * B4:(rd_slot + 1) * B4]
            ps = psum.tile([128, 6, B4], fp32, tag=f"ps_rec{d}")
            for m in range(6):
                for k in range(2):
                    nc.tensor.matmul(
                        ps[:, m, :],
                        lhsT=wh_sb[d][:, k, m * 128:(m + 1) * 128],
                        rhs=h_rhs[:, k, :],
                        start=(k == 0),
                        stop=(k == 1),
                    )
            z = work.tile([128, 6, B4], fp32, tag=f"z{d}")
            nc.vector.tensor_add(z, ps, zx_sb[:, :, u * B4:(u + 1) * B4])
            sio = work.tile([128, 4, B4], fp32, tag=f"sio{d}")
            nc.scalar.activation(out=sio, in_=z[:, 0:4, :], func=AF.Sigmoid)
            tj = work.tile([128, 2, B4], fp32, tag=f"tj{d}")
            nc.scalar.activation(out=tj, in_=z[:, 4:6, :], func=AF.Tanh)
            c_src, c_dst = (cA[d], cB[d]) if s % 2 == 0 else (cB[d], cA[d])
            d1 = work.tile([128, 2, B4], fp32, tag=f"d1{d}")
            nc.vector.tensor_sub(d1, tj, c_src)
            e1 = work.tile([128, 2, B4], fp32, tag=f"e1{d}")
            nc.vector.tensor_mul(e1, sio[:, 0:2, :], d1)
            nc.vector.tensor_add(c_dst, c_src, e1)
            tcn = work.tile([128, 2, B4], fp32, tag=f"tc{d}")
            nc.scalar.activation(out=tcn, in_=c_dst, func=AF.Tanh)
            nc.vector.tensor_mul(
                stg[:, :, wr_slot * B4:(wr_slot + 1) * B4], tcn, sio[:, 2:4, :])

        def recurrence(l, ydst):
            wh_sb = [wpool.tile([128, 2, 768], fp32, tag=f"wh{d}", name=f"wh{d}") for d in range(2)]
            for d in range(2):
                nc.gpsimd.dma_start(out=wh_sb[d], in_=wh_in[(l, d)][:, :, :])
            nc.gpsimd.dma_start(out=ystg[0][:, :, 0:B4], in_=h0t[:, :, :])
            nc.gpsimd.dma_start(out=ystg[1][:, :, S * B4:(S + 1) * B4], in_=h0t[:, :, :])
            nc.gpsimd.dma_start(out=cA[0], in_=c0t[:, :, :])
            nc.gpsimd.dma_start(out=cA[1], in_=c0t[:, :, :])
            SB = S * B4
            with tc.For_i(0, NIT, 1, hint_engines=tuple(nc.engines)) as i:
                zxf = io.tile([128, 6, SB], fp32, tag="zxf")
                zxb = io.tile([128, 6, SB], fp32, tag="zxb")
                fwd_off = i * SB
                bwd_off = i * (-SB) + (TB - SB)
                nc.gpsimd.dma_start(out=zxf, in_=zx_t[(l, 0)][:, :, bass.ds(fwd_off, SB)])
                nc.gpsimd.dma_start(out=zxb, in_=zx_t[(l, 1)][:, :, bass.ds(bwd_off, SB)])
                for s in range(S):
                    step(0, s, zxf, wh_sb)
                    step(1, s, zxb, wh_sb)
                nc.vector.tensor_copy(ystg[0][:, :, 0:B4], ystg[0][:, :, SB:SB + B4])
                nc.vector.tensor_copy(ystg[1][:, :, SB:SB + B4], ystg[1][:, :, 0:B4])
                nc.gpsimd.dma_start(out=ydst[:, 0:2, bass.ds(fwd_off, SB)],
                                  in_=ystg[0][:, :, B4:(S + 1) * B4])
                nc.gpsimd.dma_start(out=ydst[:, 2:4, bass.ds(bwd_off, SB)],
                                  in_=ystg[1][:, :, 0:SB])
            nc.gpsimd.dma_start(out=hn[:, 2 * l + 0, :, :], in_=ystg[0][:, :, 0:B4])
            nc.gpsimd.dma_start(out=hn[:, 2 * l + 1, :, :], in_=ystg[1][:, :, SB:SB + B4])
            nc.gpsimd.dma_start(out=cn[:, 2 * l + 0, :, :], in_=cA[0])
            nc.gpsimd.dma_start(out=cn[:, 2 * l + 1, :, :], in_=cA[1])

        def highway(ysrc, prev, dst):
            for nb in range(NBLK):
                yblk = io.tile([128, 4, 512], fp32, tag="yblk")
                pblk = io.tile([128, 4, 512], fp32, tag="pblk")
                nc.gpsimd.dma_start(out=yblk, in_=ysrc[:, :, nb * 512:(nb + 1) * 512])
                nc.gpsimd.dma_start(out=pblk, in_=prev[:, :, nb * 512:(nb + 1) * 512])
                for mc in range(4):
                    ps = psum.tile([128, 512], fp32, tag="ps_big")
                    for k in range(4):
                        nc.tensor.matmul(
                            ps,
                            lhsT=whw_sb[:, k, mc * 128:(mc + 1) * 128],
                            rhs=yblk[:, k, :],
                            start=(k == 0),
                            stop=(k == 3),
                        )
                    g = work.tile([128, 512], fp32, tag="g_hw")
                    nc.scalar.activation(out=g, in_=ps, func=AF.Sigmoid,
                                         bias=bhw_sb[:, mc:mc + 1])
                    d1 = work.tile([128, 512], fp32, tag="d_hw")
                    nc.vector.tensor_sub(d1, yblk[:, mc, :], pblk[:, mc, :])
                    e1 = work.tile([128, 512], fp32, tag="e_hw")
                    nc.vector.tensor_mul(e1, g, d1)
                    o1 = work.tile([128, 512], fp32, tag="o_hw")
                    nc.vector.tensor_add(o1, pblk[:, mc, :], e1)
                    nc.gpsimd.dma_start(out=dst[:, mc, nb * 512:(nb + 1) * 512], in_=o1)

        # ---------------- phase sequence ----------------
        precompute(0, xt)
        recurrence(0, yr[0])
        precompute(1, yr[0])
        recurrence(1, yr[1])
        highway(yr[1], yr[0], ocur)
        precompute(2, ocur)
        recurrence(2, yr[2])
        highway(yr[2], ocur, out2)

    nc.compile()
    return nc


def _prep_inputs(x, h0, c0, Ws, bs, W_hw, b_hw):
    """Host-side: build per-core input maps."""
    perm = [0, 1, 4, 5, 2, 3]  # gate column chunk order [i0 i1 o0 o1 j0 j1]
    common = {}
    for l in range(L):
        din = DIN if l == 0 else 2 * H
        kc = din // 128
        for d in range(2):
            W = Ws[(l, d)]
            b = bs[(l, d)]
            Wp = np.concatenate([W[:, pm * 128:(pm + 1) * 128] for pm in perm], axis=1)
            bp = np.concatenate([b[pm * 128:(pm + 1) * 128] for pm in perm])
            Wx = Wp[:din]         # [din, 768]
            Wh = Wp[din:din + H]  # [256, 768]
            common[f"wx{l}{d}"] = np.ascontiguousarray(
                Wx.reshape(kc, 128, 768).transpose(1, 0, 2))
            common[f"wh{l}{d}"] = np.ascontiguousarray(
                Wh.reshape(2, 128, 768).transpose(1, 0, 2))
            common[f"bias{l}{d}"] = np.ascontiguousarray(bp.reshape(6, 128).T)
    common["whw"] = np.ascontiguousarray(W_hw.reshape(4, 128, 512).transpose(1, 0, 2))
    common["bhw"] = np.ascontiguousarray(b_hw.reshape(4, 128).T)
    h0r = np.repeat(h0.reshape(2, 128).transpose(1, 0)[:, :, None], B4, axis=2)
    c0r = np.repeat(c0.reshape(2, 128).transpose(1, 0)[:, :, None], B4, axis=2)
    common["h0t"] = np.ascontiguousarray(h0r)   # [128, 2, B4]
    common["c0t"] = np.ascontiguousarray(c0r)

    in_maps = []
    for ci in range(NCORES):
        xs = x[:, ci * B4:(ci + 1) * B4, :]                  # [T, B4, 256]
        xtc = xs.transpose(2, 0, 1).reshape(2, 128, TB)      # [2,128,TB]
        m = dict(common)
        m["xt"] = np.ascontiguousarray(xtc.transpose(1, 0, 2))
        in_maps.append(m)
    return in_maps


def kernel(x, h0, c0, Wf0, bf0, Wb0, bb0, Wf1, bf1, Wb1, bb1, Wf2, bf2, Wb2, bb2,
           W_hw, b_hw):
    from concourse.bass_utils import run_bass_kernel_spmd

    args = dict(x=x, h0=h0, c0=c0, Wf0=Wf0, bf0=bf0, Wb0=Wb0, bb0=bb0,
                Wf1=Wf1, bf1=bf1, Wb1=Wb1, bb1=bb1, Wf2=Wf2, bf2=bf2,
                Wb2=Wb2, bb2=bb2, W_hw=W_hw, b_hw=b_hw)
    args = {k: np.asarray(v, dtype=np.float32) for k, v in args.items()}
    Ws = {(0, 0): args["Wf0"], (0, 1): args["Wb0"],
          (1, 0): args["Wf1"], (1, 1): args["Wb1"],
          (2, 0): args["Wf2"], (2, 1): args["Wb2"]}
    bs = {(0, 0): args["bf0"], (0, 1): args["bb0"],
          (1, 0): args["bf1"], (1, 1): args["bb1"],
          (2, 0): args["bf2"], (2, 1): args["bb2"]}

    if "nc" not in _CACHE:
        _CACHE["nc"] = _build()
    nc = _CACHE["nc"]

    in_maps = _prep_inputs(args["x"], args["h0"], args["c0"], Ws, bs,
                           args["W_hw"], args["b_hw"])
    res = run_bass_kernel_spmd(nc, in_maps, core_ids=list(range(NCORES)),
                               trace=TRACE)
    _CACHE["last_result"] = res

    outs, hns, cns = [], [], []
    for r in res.results:
        o = r["out2"].reshape(128, 4, T, B4).transpose(2, 3, 1, 0).reshape(T, B4, 512)
        outs.append(o)
        hns.append(r["hn"].transpose(1, 3, 2, 0).reshape(6, B4, 256))
        cns.append(r["cn"].transpose(1, 3, 2, 0).reshape(6, B4, 256))
    out = np.concatenate(outs, axis=1)
    h_n = np.concatenate(hns, axis=1)
    c_n = np.concatenate(cns, axis=1)
    return out, h_n, c_n


# revision 13
# speedup vs baseline: 7.7425x; 7.7425x over previous
"""BiLSTM (3-layer, custom 3-gate cell, highway) Trainium2 Bass kernel.

Sharding: data-parallel over batch B=32 -> 4 per core across 8 cores.
Each core runs the full 3-layer bidirectional scan on its batch shard.

Layouts (per core, B4 = 4 batch lanes):
  - Transposed activations: [128 partitions, chunk, T*B4] with the feature
    dim on partitions (chunks of 128) and (t, b) in the free dim, b inner.
  - Recurrent matmuls are weight-stationary: lhsT = Wh chunk [K=128, M=128],
    rhs = h^T [128, B4] -> psum z^T [128 (z-chunk), B4].
  - Gate column chunks permuted host-side to [i0 i1 o0 o1 j0 j1] so sigmoid
    covers one contiguous [128, 4, B4] slab and tanh covers [128, 2, B4].
"""

import numpy as np

T, BFULL, DIN, H, L = 1024, 32, 256, 256, 3
NCORES = 8
B4 = BFULL // NCORES          # 4 batch lanes per core
TB = T * B4                   # 4096 free-dim elements
S = 32                        # timesteps per For_i iteration
NIT = T // S                  # 32 iterations
NBLK = TB // 512              # 8 blocks of 512 for dense matmuls

_CACHE = {}
TRACE = False
REP = 1   # >1: benchmark mode (recurrence runs REP x longer, results invalid)


def _build():
    import concourse.bass as bass
    import concourse.bacc as bacc
    import concourse.mybir as mybir
    import concourse.tile as tile

    fp32 = mybir.dt.float32
    AF = mybir.ActivationFunctionType

    nc = bacc.Bacc("TRN2", target_bir_lowering=False, debug=False,
                   num_devices=NCORES)

    # ---------------- I/O declarations ----------------
    xt = nc.dram_tensor("xt", [128, 2, TB], fp32, kind="ExternalInput")
    h0t = nc.dram_tensor("h0t", [128, 2, B4], fp32, kind="ExternalInput")
    c0t = nc.dram_tensor("c0t", [128, 2, B4], fp32, kind="ExternalInput")
    wx_in, wh_in, bias_in = {}, {}, {}
    for l in range(L):
        kc = 2 if l == 0 else 4
        for d in range(2):
            wx_in[(l, d)] = nc.dram_tensor(f"wx{l}{d}", [128, kc, 768], fp32, kind="ExternalInput")
            wh_in[(l, d)] = nc.dram_tensor(f"wh{l}{d}", [128, 2, 768], fp32, kind="ExternalInput")
            bias_in[(l, d)] = nc.dram_tensor(f"bias{l}{d}", [128, 6], fp32, kind="ExternalInput")
    whw_in = nc.dram_tensor("whw", [128, 4, 512], fp32, kind="ExternalInput")
    bhw_in = nc.dram_tensor("bhw", [128, 4], fp32, kind="ExternalInput")

    out2 = nc.dram_tensor("out2", [128, 4, TB], fp32, kind="ExternalOutput")
    hn = nc.dram_tensor("hn", [128, 6, 2, B4], fp32, kind="ExternalOutput")
    cn = nc.dram_tensor("cn", [128, 6, 2, B4], fp32, kind="ExternalOutput")

    from contextlib import ExitStack
    ctx = ExitStack()
    with tile.TileContext(nc) as tc, ctx:
        wpool = ctx.enter_context(tc.tile_pool(name="weights", bufs=1))
        state = ctx.enter_context(tc.tile_pool(name="state", bufs=1))
        io = ctx.enter_context(tc.tile_pool(name="io", bufs=2))
        work = ctx.enter_context(tc.tile_pool(name="work", bufs=4))
        psum = ctx.enter_context(tc.tile_pool(name="psum", bufs=2, space="PSUM"))
        dram = ctx.enter_context(tc.tile_pool(name="dram", bufs=1, space="DRAM"))

        # DRAM scratch (distinct tags -> no aliasing; deps tracked via pool)
        zx_t = {(l, d): dram.tile([128, 6, TB], fp32, tag=f"zx{l}{d}", name=f"zx{l}{d}")
                for l in range(L) for d in range(2)}
        yr = [dram.tile([128, 4, TB], fp32, tag=f"yr{l}", name=f"yr{l}") for l in range(L)]
        ocur = dram.tile([128, 4, TB], fp32, tag="ocur")

        # ------------- persistent state tiles -------------
        # ystg: [p, dir, k-chunk, slot*B4]. fwd carry at slot 0 (writes 1..S
        # ascending); bwd carry at slot S (writes S-1..0 descending).
        ystg = state.tile([128, 2, 2, (S + 1) * B4], fp32, tag="ystg", name="ystg")
        cA = state.tile([128, 2, 2, B4], fp32, tag="cA", name="cA")
        cB = state.tile([128, 2, 2, B4], fp32, tag="cB", name="cB")

        def fuse_dirs(a0, a1):
            """Combine two same-pattern APs (dir 0/1) into one [p, 2, ...] AP."""
            return bass.AP(tensor=a0.tensor, offset=a0.offset,
                           ap=[list(a0.ap[0]), [a1.offset - a0.offset, 2]]
                              + [list(x) for x in a0.ap[1:]])
        bias_sb = {}
        for l in range(L):
            for d in range(2):
                t = wpool.tile([128, 6], fp32, tag=f"bias{l}{d}")
                nc.gpsimd.dma_start(out=t, in_=bias_in[(l, d)][:, :])
                bias_sb[(l, d)] = t
        bhw_sb = wpool.tile([128, 4], fp32, tag="bhw")
        nc.gpsimd.dma_start(out=bhw_sb, in_=bhw_in[:, :])
        whw_sb = wpool.tile([128, 4, 512], fp32, tag="whw")
        nc.gpsimd.dma_start(out=whw_sb, in_=whw_in[:, :, :])

        def precompute(l, src):
            """zx[l][d][:, m, :] = (Wx_d^T @ src)[m-chunk] + bias."""
            kc = 2 if l == 0 else 4
            wx_sb = [wpool.tile([128, 4, 768], fp32, tag=f"wx{d}", name=f"wx{d}") for d in range(2)]
            for d in range(2):
                nc.gpsimd.dma_start(out=wx_sb[d][:, 0:kc, :], in_=wx_in[(l, d)][:, :, :])
            for nb in range(NBLK):
                xblk = io.tile([128, 4, 512], fp32, tag="xblk")
                nc.gpsimd.dma_start(out=xblk[:, 0:kc, :],
                                  in_=src[:, 0:kc, nb * 512:(nb + 1) * 512])
                for d in range(2):
                    for m in range(6):
                        ps = psum.tile([128, 512], fp32, tag="ps_big")
                        for k in range(kc):
                            nc.tensor.matmul(
                                ps,
                                lhsT=wx_sb[d][:, k, m * 128:(m + 1) * 128],
                                rhs=xblk[:, k, :],
                                start=(k == 0),
                                stop=(k == kc - 1),
                            )
                        zs = work.tile([128, 512], fp32, tag="zs_pre")
                        nc.scalar.activation(
                            out=zs, in_=ps, func=AF.Identity,
                            bias=bias_sb[(l, d)][:, m:m + 1],
                        )
                        nc.gpsimd.dma_start(
                            out=zx_t[(l, d)][:, m, nb * 512:(nb + 1) * 512], in_=zs)

        def step(s, zx_sb, wh_sb):
            """One timestep of BOTH directions, gate math merged into wide ops."""
            rd = (s, S - s)          # read slots (fwd, bwd)
            wr = (s + 1, S - 1 - s)  # write slots
            u = (s, S - 1 - s)       # zx time index within block
            ps = psum.tile([128, 2, 6, B4], fp32, tag="ps_rec")
            for d in range(2):
                for m in range(6):
                    for k in range(2):
                        nc.tensor.matmul(
                            ps[:, d, m, :],
                            lhsT=wh_sb[d][:, k, m * 128:(m + 1) * 128],
                            rhs=ystg[:, d, k, rd[d] * B4:(rd[d] + 1) * B4],
                            start=(k == 0),
                            stop=(k == 1),
                        )
            z = work.tile([128, 2, 6, B4], fp32, tag="z")
            zx_ap = fuse_dirs(zx_sb[:, 0, :, u[0] * B4:(u[0] + 1) * B4],
                              zx_sb[:, 1, :, u[1] * B4:(u[1] + 1) * B4])
            nc.vector.tensor_add(z, ps, zx_ap)
            sio = work.tile([128, 2, 4, B4], fp32, tag="sio")
            nc.scalar.activation(out=sio, in_=z[:, :, 0:4, :], func=AF.Sigmoid)
            tj = work.tile([128, 2, 2, B4], fp32, tag="tj")
            nc.scalar.activation(out=tj, in_=z[:, :, 4:6, :], func=AF.Tanh)
            c_src, c_dst = (cA, cB) if s % 2 == 0 else (cB, cA)
            d1 = work.tile([128, 2, 2, B4], fp32, tag="d1")
            nc.vector.tensor_sub(d1, tj, c_src)
            e1 = work.tile([128, 2, 2, B4], fp32, tag="e1")
            nc.vector.tensor_mul(e1, sio[:, :, 0:2, :], d1)
            nc.vector.tensor_add(c_dst, c_src, e1)
            tcn = work.tile([128, 2, 2, B4], fp32, tag="tc")
            nc.scalar.activation(out=tcn, in_=c_dst, func=AF.Tanh)
            y_out = fuse_dirs(ystg[:, 0, :, wr[0] * B4:(wr[0] + 1) * B4],
                              ystg[:, 1, :, wr[1] * B4:(wr[1] + 1) * B4])
            nc.vector.tensor_mul(y_out, tcn, sio[:, :, 2:4, :])

        def recurrence(l, ydst):
            wh_sb = [wpool.tile([128, 2, 768], fp32, tag=f"wh{d}", name=f"wh{d}") for d in range(2)]
            for d in range(2):
                nc.gpsimd.dma_start(out=wh_sb[d], in_=wh_in[(l, d)][:, :, :])
            SB = S * B4
            nc.gpsimd.dma_start(out=ystg[:, 0, :, 0:B4], in_=h0t[:, :, :])
            nc.gpsimd.dma_start(out=ystg[:, 1, :, SB:SB + B4], in_=h0t[:, :, :])
            nc.gpsimd.dma_start(out=cA[:, 0, :, :], in_=c0t[:, :, :])
            nc.gpsimd.dma_start(out=cA[:, 1, :, :], in_=c0t[:, :, :])
            with tc.For_i(0, NIT * REP, 1, hint_engines=tuple(nc.engines)) as i:
                zx = io.tile([128, 2, 6, SB], fp32, tag="zx")
                if REP == 1:
                    fwd_off = i * SB
                    bwd_off = i * (-SB) + (TB - SB)
                else:  # bench mode: fixed offsets, same work per iteration
                    fwd_off = 0
                    bwd_off = TB - SB
                nc.gpsimd.dma_start(out=zx[:, 0, :, :],
                                    in_=zx_t[(l, 0)][:, :, bass.ds(fwd_off, SB)])
                nc.gpsimd.dma_start(out=zx[:, 1, :, :],
                                    in_=zx_t[(l, 1)][:, :, bass.ds(bwd_off, SB)])
                for s in range(S):
                    step(s, zx, wh_sb)
                # move carries: fwd slot S -> 0, bwd slot 0 -> S (one wide op)
                carry_out = fuse_dirs(ystg[:, 0, :, 0:B4], ystg[:, 1, :, SB:SB + B4])
                carry_in = fuse_dirs(ystg[:, 0, :, SB:SB + B4], ystg[:, 1, :, 0:B4])
                nc.vector.tensor_copy(carry_out, carry_in)
                nc.gpsimd.dma_start(out=ydst[:, 0:2, bass.ds(fwd_off, SB)],
                                    in_=ystg[:, 0, :, B4:(S + 1) * B4])
                nc.gpsimd.dma_start(out=ydst[:, 2:4, bass.ds(bwd_off, SB)],
                                    in_=ystg[:, 1, :, 0:SB])
            nc.gpsimd.dma_start(out=hn[:, 2 * l + 0, :, :], in_=ystg[:, 0, :, 0:B4])
            nc.gpsimd.dma_start(out=hn[:, 2 * l + 1, :, :], in_=ystg[:, 1, :, SB:SB + B4])
            nc.gpsimd.dma_start(out=cn[:, 2 * l + 0, :, :], in_=cA[:, 0, :, :])
            nc.gpsimd.dma_start(out=cn[:, 2 * l + 1, :, :], in_=cA[:, 1, :, :])

        def highway(ysrc, prev, dst):
            for nb in range(NBLK):
                yblk = io.tile([128, 4, 512], fp32, tag="yblk")
                pblk = io.tile([128, 4, 512], fp32, tag="pblk")
                nc.gpsimd.dma_start(out=yblk, in_=ysrc[:, :, nb * 512:(nb + 1) * 512])
                nc.gpsimd.dma_start(out=pblk, in_=prev[:, :, nb * 512:(nb + 1) * 512])
                for mc in range(4):
                    ps = psum.tile([128, 512], fp32, tag="ps_big")
                    for k in range(4):
                        nc.tensor.matmul(
                            ps,
                            lhsT=whw_sb[:, k, mc * 128:(mc + 1) * 128],
                            rhs=yblk[:, k, :],
                            start=(k == 0),
                            stop=(k == 3),
                        )
                    g = work.tile([128, 512], fp32, tag="g_hw")
                    nc.scalar.activation(out=g, in_=ps, func=AF.Sigmoid,
                                         bias=bhw_sb[:, mc:mc + 1])
                    d1 = work.tile([128, 512], fp32, tag="d_hw")
                    nc.vector.tensor_sub(d1, yblk[:, mc, :], pblk[:, mc, :])
                    e1 = work.tile([128, 512], fp32, tag="e_hw")
                    nc.vector.tensor_mul(e1, g, d1)
                    o1 = work.tile([128, 512], fp32, tag="o_hw")
                    nc.vector.tensor_add(o1, pblk[:, mc, :], e1)
                    nc.gpsimd.dma_start(out=dst[:, mc, nb * 512:(nb + 1) * 512], in_=o1)

        # ---------------- phase sequence ----------------
        precompute(0, xt)
        recurrence(0, yr[0])
        precompute(1, yr[0])
        recurrence(1, yr[1])
        highway(yr[1], yr[0], ocur)
        precompute(2, ocur)
        recurrence(2, yr[2])
        highway(yr[2], ocur, out2)

    nc.compile()
    return nc


def _prep_inputs(x, h0, c0, Ws, bs, W_hw, b_hw):
    """Host-side: build per-core input maps."""
    perm = [0, 1, 4, 5, 2, 3]  # gate column chunk order [i0 i1 o0 o1 j0 j1]
    common = {}
    for l in range(L):
        din = DIN if l == 0 else 2 * H
        kc = din // 128
        for d in range(2):
            W = Ws[(l, d)]
            b = bs[(l, d)]
            Wp = np.concatenate([W[:, pm * 128:(pm + 1) * 128] for pm in perm], axis=1)
            bp = np.concatenate([b[pm * 128:(pm + 1) * 128] for pm in perm])
            Wx = Wp[:din]         # [din, 768]
            Wh = Wp[din:din + H]  # [256, 768]
            common[f"wx{l}{d}"] = np.ascontiguousarray(
                Wx.reshape(kc, 128, 768).transpose(1, 0, 2))
            common[f"wh{l}{d}"] = np.ascontiguousarray(
                Wh.reshape(2, 128, 768).transpose(1, 0, 2))
            common[f"bias{l}{d}"] = np.ascontiguousarray(bp.reshape(6, 128).T)
    common["whw"] = np.ascontiguousarray(W_hw.reshape(4, 128, 512).transpose(1, 0, 2))
    common["bhw"] = np.ascontiguousarray(b_hw.reshape(4, 128).T)
    h0r = np.repeat(h0.reshape(2, 128).transpose(1, 0)[:, :, None], B4, axis=2)
    c0r = np.repeat(c0.reshape(2, 128).transpose(1, 0)[:, :, None], B4, axis=2)
    common["h0t"] = np.ascontiguousarray(h0r)   # [128, 2, B4]
    common["c0t"] = np.ascontiguousarray(c0r)

    in_maps = []
    for ci in range(NCORES):
        xs = x[:, ci * B4:(ci + 1) * B4, :]                  # [T, B4, 256]
        xtc = xs.transpose(2, 0, 1).reshape(2, 128, TB)      # [2,128,TB]
        m = dict(common)
        m["xt"] = np.ascontiguousarray(xtc.transpose(1, 0, 2))
        in_maps.append(m)
    return in_maps


def kernel(x, h0, c0, Wf0, bf0, Wb0, bb0, Wf1, bf1, Wb1, bb1, Wf2, bf2, Wb2, bb2,
           W_hw, b_hw):
    from concourse.bass_utils import run_bass_kernel_spmd

    args = dict(x=x, h0=h0, c0=c0, Wf0=Wf0, bf0=bf0, Wb0=Wb0, bb0=bb0,
                Wf1=Wf1, bf1=bf1, Wb1=Wb1, bb1=bb1, Wf2=Wf2, bf2=bf2,
                Wb2=Wb2, bb2=bb2, W_hw=W_hw, b_hw=b_hw)
    args = {k: np.asarray(v, dtype=np.float32) for k, v in args.items()}
    Ws = {(0, 0): args["Wf0"], (0, 1): args["Wb0"],
          (1, 0): args["Wf1"], (1, 1): args["Wb1"],
          (2, 0): args["Wf2"], (2, 1): args["Wb2"]}
    bs = {(0, 0): args["bf0"], (0, 1): args["bb0"],
          (1, 0): args["bf1"], (1, 1): args["bb1"],
          (2, 0): args["bf2"], (2, 1): args["bb2"]}

    if "nc" not in _CACHE:
        _CACHE["nc"] = _build()
    nc = _CACHE["nc"]

    in_maps = _prep_inputs(args["x"], args["h0"], args["c0"], Ws, bs,
                           args["W_hw"], args["b_hw"])
    res = run_bass_kernel_spmd(nc, in_maps, core_ids=list(range(NCORES)),
                               trace=TRACE)
    _CACHE["last_result"] = res

    outs, hns, cns = [], [], []
    for r in res.results:
        o = r["out2"].reshape(128, 4, T, B4).transpose(2, 3, 1, 0).reshape(T, B4, 512)
        outs.append(o)
        hns.append(r["hn"].transpose(1, 3, 2, 0).reshape(6, B4, 256))
        cns.append(r["cn"].transpose(1, 3, 2, 0).reshape(6, B4, 256))
    out = np.concatenate(outs, axis=1)
    h_n = np.concatenate(hns, axis=1)
    c_n = np.concatenate(cns, axis=1)
    return out, h_n, c_n


# revision 16
# speedup vs baseline: 8.8248x; 1.1398x over previous
"""BiLSTM (3-layer, custom 3-gate cell, highway) Trainium2 Bass kernel.

Sharding: data-parallel over batch B=32 -> 4 per core across 8 cores.
Each core runs the full 3-layer bidirectional scan on its batch shard.

Layouts (per core, B4 = 4 batch lanes):
  - Transposed activations: [128 partitions, chunk, T*B4] with the feature
    dim on partitions (chunks of 128) and (t, b) in the free dim, b inner.
  - Recurrent matmuls are weight-stationary: lhsT = Wh chunk [K=128, M=128],
    rhs = h^T [128, B4] -> psum z^T [128 (z-chunk), B4].
  - Gate column chunks permuted host-side to [i0 i1 o0 o1 j0 j1] so sigmoid
    covers one contiguous [128, 4, B4] slab and tanh covers [128, 2, B4].
"""

import numpy as np

T, BFULL, DIN, H, L = 1024, 32, 256, 256, 3
NCORES = 8
B4 = BFULL // NCORES          # 4 batch lanes per core
TB = T * B4                   # 4096 free-dim elements
S = 32                        # timesteps per For_i iteration
NIT = T // S                  # 32 iterations
NBLK = TB // 512              # 8 blocks of 512 for dense matmuls

_CACHE = {}
TRACE = False
REP = 1   # >1: benchmark mode (recurrence runs REP x longer, results invalid)
PROBE = 0  # bench-only: 1=half matmuls, 2=skip gate math, 3=skip matmuls


def _build():
    import concourse.bass as bass
    import concourse.bacc as bacc
    import concourse.mybir as mybir
    import concourse.tile as tile

    fp32 = mybir.dt.float32
    AF = mybir.ActivationFunctionType

    nc = bacc.Bacc("TRN2", target_bir_lowering=False, debug=False,
                   num_devices=NCORES)

    # ---------------- I/O declarations ----------------
    xt = nc.dram_tensor("xt", [128, 2, TB], fp32, kind="ExternalInput")
    h0t = nc.dram_tensor("h0t", [128, 2, B4], fp32, kind="ExternalInput")
    c0t = nc.dram_tensor("c0t", [128, 2, B4], fp32, kind="ExternalInput")
    wx_in, wh_in, bias_in = {}, {}, {}
    for l in range(L):
        kc = 2 if l == 0 else 4
        for d in range(2):
            wx_in[(l, d)] = nc.dram_tensor(f"wx{l}{d}", [128, kc, 768], fp32, kind="ExternalInput")
            wh_in[(l, d)] = nc.dram_tensor(f"wh{l}{d}", [128, 2, 768], fp32, kind="ExternalInput")
            bias_in[(l, d)] = nc.dram_tensor(f"bias{l}{d}", [128, 6], fp32, kind="ExternalInput")
    whw_in = nc.dram_tensor("whw", [128, 4, 512], fp32, kind="ExternalInput")
    bhw_in = nc.dram_tensor("bhw", [128, 4], fp32, kind="ExternalInput")

    out2 = nc.dram_tensor("out2", [128, 4, TB], fp32, kind="ExternalOutput")
    hn = nc.dram_tensor("hn", [128, 6, 2, B4], fp32, kind="ExternalOutput")
    cn = nc.dram_tensor("cn", [128, 6, 2, B4], fp32, kind="ExternalOutput")

    from contextlib import ExitStack
    ctx = ExitStack()
    with tile.TileContext(nc) as tc, ctx:
        wpool = ctx.enter_context(tc.tile_pool(name="weights", bufs=1))
        state = ctx.enter_context(tc.tile_pool(name="state", bufs=1))
        io = ctx.enter_context(tc.tile_pool(name="io", bufs=2))
        work = ctx.enter_context(tc.tile_pool(name="work", bufs=4))
        psum = ctx.enter_context(tc.tile_pool(name="psum", bufs=2, space="PSUM"))
        dram = ctx.enter_context(tc.tile_pool(name="dram", bufs=1, space="DRAM"))

        # DRAM scratch (distinct tags -> no aliasing; deps tracked via pool)
        zx_t = {(l, d): dram.tile([128, 6, TB], fp32, tag=f"zx{l}{d}", name=f"zx{l}{d}")
                for l in range(L) for d in range(2)}
        yr = [dram.tile([128, 4, TB], fp32, tag=f"yr{l}", name=f"yr{l}") for l in range(L)]
        ocur = dram.tile([128, 4, TB], fp32, tag="ocur")

        # ------------- persistent state tiles -------------
        # ystg: [p, dir, k-chunk, slot*B4]. fwd carry at slot 0 (writes 1..S
        # ascending); bwd carry at slot S (writes S-1..0 descending).
        ystg = state.tile([128, 2, 2, (S + 1) * B4], fp32, tag="ystg", name="ystg")
        cA = state.tile([128, 2, 2, B4], fp32, tag="cA", name="cA")
        cB = state.tile([128, 2, 2, B4], fp32, tag="cB", name="cB")

        def fuse_dirs(a0, a1):
            """Combine two same-pattern APs (dir 0/1) into one [p, 2, ...] AP."""
            return bass.AP(tensor=a0.tensor, offset=a0.offset,
                           ap=[list(a0.ap[0]), [a1.offset - a0.offset, 2]]
                              + [list(x) for x in a0.ap[1:]])
        bias_sb = {}
        for l in range(L):
            for d in range(2):
                t = wpool.tile([128, 6], fp32, tag=f"bias{l}{d}")
                nc.gpsimd.dma_start(out=t, in_=bias_in[(l, d)][:, :])
                bias_sb[(l, d)] = t
        bhw_sb = wpool.tile([128, 4], fp32, tag="bhw")
        nc.gpsimd.dma_start(out=bhw_sb, in_=bhw_in[:, :])
        whw_sb = wpool.tile([128, 4, 512], fp32, tag="whw")
        nc.gpsimd.dma_start(out=whw_sb, in_=whw_in[:, :, :])

        def precompute(l, src):
            """zx[l][d][:, m, :] = (Wx_d^T @ src)[m-chunk] + bias."""
            kc = 2 if l == 0 else 4
            wx_sb = [wpool.tile([128, 4, 768], fp32, tag=f"wx{d}", name=f"wx{d}") for d in range(2)]
            for d in range(2):
                nc.gpsimd.dma_start(out=wx_sb[d][:, 0:kc, :], in_=wx_in[(l, d)][:, :, :])
            for nb in range(NBLK):
                xblk = io.tile([128, 4, 512], fp32, tag="xblk")
                nc.gpsimd.dma_start(out=xblk[:, 0:kc, :],
                                  in_=src[:, 0:kc, nb * 512:(nb + 1) * 512])
                for d in range(2):
                    for m in range(6):
                        ps = psum.tile([128, 512], fp32, tag="ps_big")
                        for k in range(kc):
                            nc.tensor.matmul(
                                ps,
                                lhsT=wx_sb[d][:, k, m * 128:(m + 1) * 128],
                                rhs=xblk[:, k, :],
                                start=(k == 0),
                                stop=(k == kc - 1),
                            )
                        zs = work.tile([128, 512], fp32, tag="zs_pre")
                        nc.scalar.activation(
                            out=zs, in_=ps, func=AF.Identity,
                            bias=bias_sb[(l, d)][:, m:m + 1],
                        )
                        nc.gpsimd.dma_start(
                            out=zx_t[(l, d)][:, m, nb * 512:(nb + 1) * 512], in_=zs)

        def step(s, zx_sb, wh_sb):
            """One timestep of BOTH directions, gate math merged into wide ops."""
            rd = (s, S - s)          # read slots (fwd, bwd)
            wr = (s + 1, S - 1 - s)  # write slots
            u = (s, S - 1 - s)       # zx time index within block
            y_out = fuse_dirs(ystg[:, 0, :, wr[0] * B4:(wr[0] + 1) * B4],
                              ystg[:, 1, :, wr[1] * B4:(wr[1] + 1) * B4])
            zx_io = fuse_dirs(zx_sb[:, 0, 0:4, u[0] * B4:(u[0] + 1) * B4],
                              zx_sb[:, 1, 0:4, u[1] * B4:(u[1] + 1) * B4])
            zx_j = fuse_dirs(zx_sb[:, 0, 4:6, u[0] * B4:(u[0] + 1) * B4],
                             zx_sb[:, 1, 4:6, u[1] * B4:(u[1] + 1) * B4])

            def mm(ps, d, mm_i, m):
                for k in range(2):
                    nc.tensor.matmul(
                        ps[:, d, mm_i, :],
                        lhsT=wh_sb[d][:, k, m * 128:(m + 1) * 128],
                        rhs=ystg[:, d, k, rd[d] * B4:(rd[d] + 1) * B4],
                        start=(k == 0),
                        stop=(k == 1),
                    )

            # j-gate matmuls first: the tanh/cell chain overlaps the i/o MMs.
            ps_j = psum.tile([128, 2, 2, B4], fp32, tag="ps_j")
            for d in range(2):
                for j, m in enumerate((4, 5)):
                    mm(ps_j, d, j, m)
            ps_io = psum.tile([128, 2, 4, B4], fp32, tag="ps_io")
            for d in range(2):
                for m in range(4):
                    mm(ps_io, d, m, m)
            zj = work.tile([128, 2, 2, B4], fp32, tag="zj")
            nc.vector.tensor_add(zj, ps_j, zx_j)
            tj = work.tile([128, 2, 2, B4], fp32, tag="tj")
            nc.scalar.activation(out=tj, in_=zj, func=AF.Tanh)
            c_src, c_dst = (cA, cB) if s % 2 == 0 else (cB, cA)
            d1 = work.tile([128, 2, 2, B4], fp32, tag="d1")
            nc.vector.tensor_sub(d1, tj, c_src)
            zio = work.tile([128, 2, 4, B4], fp32, tag="zio")
            nc.vector.tensor_add(zio, ps_io, zx_io)
            sio = work.tile([128, 2, 4, B4], fp32, tag="sio")
            nc.scalar.activation(out=sio, in_=zio, func=AF.Sigmoid)
            e1 = work.tile([128, 2, 2, B4], fp32, tag="e1")
            nc.vector.tensor_mul(e1, sio[:, :, 0:2, :], d1)
            nc.vector.tensor_add(c_dst, c_src, e1)
            tcn = work.tile([128, 2, 2, B4], fp32, tag="tc")
            nc.scalar.activation(out=tcn, in_=c_dst, func=AF.Tanh)
            nc.vector.tensor_mul(y_out, tcn, sio[:, :, 2:4, :])

        def recurrence(l, ydst):
            wh_sb = [wpool.tile([128, 2, 768], fp32, tag=f"wh{d}", name=f"wh{d}") for d in range(2)]
            for d in range(2):
                nc.gpsimd.dma_start(out=wh_sb[d], in_=wh_in[(l, d)][:, :, :])
            SB = S * B4
            nc.gpsimd.dma_start(out=ystg[:, 0, :, 0:B4], in_=h0t[:, :, :])
            nc.gpsimd.dma_start(out=ystg[:, 1, :, SB:SB + B4], in_=h0t[:, :, :])
            nc.gpsimd.dma_start(out=cA[:, 0, :, :], in_=c0t[:, :, :])
            nc.gpsimd.dma_start(out=cA[:, 1, :, :], in_=c0t[:, :, :])
            with tc.For_i(0, NIT * REP, 1, hint_engines=tuple(nc.engines)) as i:
                zx = io.tile([128, 2, 6, SB], fp32, tag="zx")
                if REP == 1:
                    fwd_off = i * SB
                    bwd_off = i * (-SB) + (TB - SB)
                else:  # bench mode: fixed offsets, same work per iteration
                    fwd_off = 0
                    bwd_off = TB - SB
                nc.gpsimd.dma_start(out=zx[:, 0, :, :],
                                    in_=zx_t[(l, 0)][:, :, bass.ds(fwd_off, SB)])
                nc.gpsimd.dma_start(out=zx[:, 1, :, :],
                                    in_=zx_t[(l, 1)][:, :, bass.ds(bwd_off, SB)])
                for s in range(S):
                    step(s, zx, wh_sb)
                # move carries: fwd slot S -> 0, bwd slot 0 -> S (one wide op)
                carry_out = fuse_dirs(ystg[:, 0, :, 0:B4], ystg[:, 1, :, SB:SB + B4])
                carry_in = fuse_dirs(ystg[:, 0, :, SB:SB + B4], ystg[:, 1, :, 0:B4])
                nc.vector.tensor_copy(carry_out, carry_in)
                nc.gpsimd.dma_start(out=ydst[:, 0:2, bass.ds(fwd_off, SB)],
                                    in_=ystg[:, 0, :, B4:(S + 1) * B4])
                nc.gpsimd.dma_start(out=ydst[:, 2:4, bass.ds(bwd_off, SB)],
                                    in_=ystg[:, 1, :, 0:SB])
            nc.gpsimd.dma_start(out=hn[:, 2 * l + 0, :, :], in_=ystg[:, 0, :, 0:B4])
            nc.gpsimd.dma_start(out=hn[:, 2 * l + 1, :, :], in_=ystg[:, 1, :, SB:SB + B4])
            nc.gpsimd.dma_start(out=cn[:, 2 * l + 0, :, :], in_=cA[:, 0, :, :])
            nc.gpsimd.dma_start(out=cn[:, 2 * l + 1, :, :], in_=cA[:, 1, :, :])

        def highway(ysrc, prev, dst):
            for nb in range(NBLK):
                yblk = io.tile([128, 4, 512], fp32, tag="yblk")
                pblk = io.tile([128, 4, 512], fp32, tag="pblk")
                nc.gpsimd.dma_start(out=yblk, in_=ysrc[:, :, nb * 512:(nb + 1) * 512])
                nc.gpsimd.dma_start(out=pblk, in_=prev[:, :, nb * 512:(nb + 1) * 512])
                for mc in range(4):
                    ps = psum.tile([128, 512], fp32, tag="ps_big")
                    for k in range(4):
                        nc.tensor.matmul(
                            ps,
                            lhsT=whw_sb[:, k, mc * 128:(mc + 1) * 128],
                            rhs=yblk[:, k, :],
                            start=(k == 0),
                            stop=(k == 3),
                        )
                    g = work.tile([128, 512], fp32, tag="g_hw")
                    nc.scalar.activation(out=g, in_=ps, func=AF.Sigmoid,
                                         bias=bhw_sb[:, mc:mc + 1])
                    d1 = work.tile([128, 512], fp32, tag="d_hw")
                    nc.vector.tensor_sub(d1, yblk[:, mc, :], pblk[:, mc, :])
                    e1 = work.tile([128, 512], fp32, tag="e_hw")
                    nc.vector.tensor_mul(e1, g, d1)
                    o1 = work.tile([128, 512], fp32, tag="o_hw")
                    nc.vector.tensor_add(o1, pblk[:, mc, :], e1)
                    nc.gpsimd.dma_start(out=dst[:, mc, nb * 512:(nb + 1) * 512], in_=o1)

        # ---------------- phase sequence ----------------
        precompute(0, xt)
        recurrence(0, yr[0])
        precompute(1, yr[0])
        recurrence(1, yr[1])
        highway(yr[1], yr[0], ocur)
        precompute(2, ocur)
        recurrence(2, yr[2])
        highway(yr[2], ocur, out2)

    nc.compile()
    return nc


def _prep_inputs(x, h0, c0, Ws, bs, W_hw, b_hw):
    """Host-side: build per-core input maps."""
    perm = [0, 1, 4, 5, 2, 3]  # gate column chunk order [i0 i1 o0 o1 j0 j1]
    common = {}
    for l in range(L):
        din = DIN if l == 0 else 2 * H
        kc = din // 128
        for d in range(2):
            W = Ws[(l, d)]
            b = bs[(l, d)]
            Wp = np.concatenate([W[:, pm * 128:(pm + 1) * 128] for pm in perm], axis=1)
            bp = np.concatenate([b[pm * 128:(pm + 1) * 128] for pm in perm])
            Wx = Wp[:din]         # [din, 768]
            Wh = Wp[din:din + H]  # [256, 768]
            common[f"wx{l}{d}"] = np.ascontiguousarray(
                Wx.reshape(kc, 128, 768).transpose(1, 0, 2))
            common[f"wh{l}{d}"] = np.ascontiguousarray(
                Wh.reshape(2, 128, 768).transpose(1, 0, 2))
            common[f"bias{l}{d}"] = np.ascontiguousarray(bp.reshape(6, 128).T)
    common["whw"] = np.ascontiguousarray(W_hw.reshape(4, 128, 512).transpose(1, 0, 2))
    common["bhw"] = np.ascontiguousarray(b_hw.reshape(4, 128).T)
    h0r = np.repeat(h0.reshape(2, 128).transpose(1, 0)[:, :, None], B4, axis=2)
    c0r = np.repeat(c0.reshape(2, 128).transpose(1, 0)[:, :, None], B4, axis=2)
    common["h0t"] = np.ascontiguousarray(h0r)   # [128, 2, B4]
    common["c0t"] = np.ascontiguousarray(c0r)

    in_maps = []
    for ci in range(NCORES):
        xs = x[:, ci * B4:(ci + 1) * B4, :]                  # [T, B4, 256]
        xtc = xs.transpose(2, 0, 1).reshape(2, 128, TB)      # [2,128,TB]
        m = dict(common)
        m["xt"] = np.ascontiguousarray(xtc.transpose(1, 0, 2))
        in_maps.append(m)
    return in_maps


def kernel(x, h0, c0, Wf0, bf0, Wb0, bb0, Wf1, bf1, Wb1, bb1, Wf2, bf2, Wb2, bb2,
           W_hw, b_hw):
    from concourse.bass_utils import run_bass_kernel_spmd

    args = dict(x=x, h0=h0, c0=c0, Wf0=Wf0, bf0=bf0, Wb0=Wb0, bb0=bb0,
                Wf1=Wf1, bf1=bf1, Wb1=Wb1, bb1=bb1, Wf2=Wf2, bf2=bf2,
                Wb2=Wb2, bb2=bb2, W_hw=W_hw, b_hw=b_hw)
    args = {k: np.asarray(v, dtype=np.float32) for k, v in args.items()}
    Ws = {(0, 0): args["Wf0"], (0, 1): args["Wb0"],
          (1, 0): args["Wf1"], (1, 1): args["Wb1"],
          (2, 0): args["Wf2"], (2, 1): args["Wb2"]}
    bs = {(0, 0): args["bf0"], (0, 1): args["bb0"],
          (1, 0): args["bf1"], (1, 1): args["bb1"],
          (2, 0): args["bf2"], (2, 1): args["bb2"]}

    if "nc" not in _CACHE:
        _CACHE["nc"] = _build()
    nc = _CACHE["nc"]

    in_maps = _prep_inputs(args["x"], args["h0"], args["c0"], Ws, bs,
                           args["W_hw"], args["b_hw"])
    res = run_bass_kernel_spmd(nc, in_maps, core_ids=list(range(NCORES)),
                               trace=TRACE)
    _CACHE["last_result"] = res

    outs, hns, cns = [], [], []
    for r in res.results:
        o = r["out2"].reshape(128, 4, T, B4).transpose(2, 3, 1, 0).reshape(T, B4, 512)
        outs.append(o)
        hns.append(r["hn"].transpose(1, 3, 2, 0).reshape(6, B4, 256))
        cns.append(r["cn"].transpose(1, 3, 2, 0).reshape(6, B4, 256))
    out = np.concatenate(outs, axis=1)
    h_n = np.concatenate(hns, axis=1)
    c_n = np.concatenate(cns, axis=1)
    return out, h_n, c_n
